# revision 41
# baseline (speedup 1.0000x reference)
"""Single-head attention (B=8, T=2048, C=512, d_k=64) on 8 Trainium2 cores.

Strategy: data-parallel over batch B — one batch element per NeuronCore,
no collectives. All matmuls fp16 (1 PE cycle/row) with fp32 PSUM
accumulation; transposes are regular PE matmuls against an fp16
identity.

S^T matmuls contract over d_k=64, which would leave half the 128x128 PE
array idle, so key-tiles are processed in PAIRS packed into disjoint
row-groups (tile_position (0,0) / (64,0)) and run concurrently — this
also keeps the HAM activity monitor happy so the PE holds 2.4 GHz in
the steady state. The packing needs Q^T/K^T replicated on both
partition halves, so the projections run twice with swapped stationary
weights [Wq|Wk] / [Wk|Wq], producing QK_A = [Q^T; K^T] and
QK_B = [K^T; Q^T] at no extra matmul cost.

Softmax is split: ACT does exact exp on 56% of each score tile; DVE
writes the rest directly as fp16 bits via the Schraudolph trick
(int16(A*x+B) ~= fp16 exp; ~2% weight error on 44% of keys -> ~0.6%
output error vs the 2e-2 budget). The pair's two PSUM banks are one
contiguous tile, so each engine covers both keys of the pair in a
single strided-AP instruction.

A ones column in V makes the softmax denominator fall out of the AV
matmul. The epilogue transposes out^T back via the DMA XBAR (padded to
80 rows), normalizes with DVE reciprocal/multiply, and stores each
512-token chunk with one contiguous 128KB DMA.
"""

import numpy as np
from contextlib import ExitStack

import concourse.bass as bass
import concourse.tile as tile
from concourse import bacc
from concourse import mybir
from concourse.bass_utils import run_bass_kernel_spmd
from concourse.masks import make_identity

B, T, C, DK = 8, 2048, 512, 64
N_CORES = 8
FP32 = mybir.dt.float32
FP16 = mybir.dt.float16
I16 = mybir.dt.int16
P = 128
TT = T // P      # 16 token tiles
NP = TT // 2     # 8 key-tile pairs
CCH = C // P     # 4 contraction chunks
NB = 512         # matmul moving-operand max (PSUM bank = 512 fp32)
IC = T // NB     # 4 i-chunks
SPL = 320        # per-key ACT exp columns (of NB); rest on DVE
PAD = 80         # out^T rows padded to /16 for the DMA XBAR transpose
SCALE = 1.0 / np.sqrt(np.float32(DK))
# fp16 Schraudolph: int16(A*x + B) bitcast to fp16 approximates exp(x)
A_SCH = float((1 << 10) / np.log(2.0) * SCALE)
B_SCH = float(15 * (1 << 10) - 366393.0 / (1 << 13))

_cached = {}


def _build_nc():
    nc = bacc.Bacc("TRN2", target_bir_lowering=False, debug=False)
    x_d = nc.declare_dram_parameter("x", [T, C], FP32, isOutput=False)
    wq_d = nc.declare_dram_parameter("Wq", [C, DK], FP32, isOutput=False)
    wk_d = nc.declare_dram_parameter("Wk", [C, DK], FP32, isOutput=False)
    wv_d = nc.declare_dram_parameter("Wv", [C, DK], FP32, isOutput=False)
    out_d = nc.declare_dram_parameter("out", [T, DK], FP32, isOutput=True)

    x_g = x_d.rearrange("(g tt p) c -> g p tt c", p=P, tt=4)   # [4,128,4,512]
    out_q = out_d.rearrange("(ic p4) d -> ic p4 d", p4=4 * P)  # [4,512,64]

    with ExitStack() as ctx:
        tc = ctx.enter_context(tile.TileContext(nc))
        const = ctx.enter_context(tc.tile_pool(name="const", bufs=1))

        idb = const.tile([P, P], FP16)
        make_identity(nc, idb)

        # --- weights: fp32 staging -> fp16 packed stationaries ---
        wqk_a = const.tile([P, CCH, P], FP16)   # [Wq | Wk] per c-chunk
        wqk_b = const.tile([P, CCH, P], FP16)   # [Wk | Wq]
        wv_s = const.tile([P, CCH, DK], FP16)
        with tc.tile_pool(name="wstage", bufs=1) as wstage:
            stg = {}
            for w_d in (wq_d, wk_d, wv_d):
                w_stg = wstage.tile([P, CCH, DK], FP32, tag=f"stg{w_d.name}")
                nc.sync.dma_start(out=w_stg, in_=w_d.rearrange("(ch p) d -> p ch d", p=P))
                stg[w_d.name] = w_stg
            nc.vector.tensor_copy(out=wqk_a[:, :, 0:DK], in_=stg["Wq"])
            nc.vector.tensor_copy(out=wqk_a[:, :, DK:P], in_=stg["Wk"])
            nc.vector.tensor_copy(out=wqk_b[:, :, 0:DK], in_=stg["Wk"])
            nc.vector.tensor_copy(out=wqk_b[:, :, DK:P], in_=stg["Wq"])
            nc.vector.tensor_copy(out=wv_s, in_=stg["Wv"])

        xT = const.tile([P, CCH, T], FP16)          # x^T, 16KB/part
        v_s = const.tile([P, TT, DK + 1], FP16)     # V with ones col
        nc.vector.memset(v_s, 1.0)
        qk_a = const.tile([P, T], FP16)             # [Q^T; K^T]
        qk_b = const.tile([P, T], FP16)             # [K^T; Q^T]
        vT = const.tile([DK, T], FP16)
        # out^T staging, padded to 80 rows for the XBAR; rows 65-79 = 0
        oT80 = const.tile([PAD, IC, NB], FP16)
        nc.vector.memset(oT80, 0.0)

        # --- phase 1: 1MB x loads, DVE cast, XBAR DMA transposes into xT ---
        with tc.tile_pool(name="xbpool", bufs=1) as xbpool:
            x_all = xbpool.tile([P, IC, 4, C], FP32, tag="x_all")
            xb_all = xbpool.tile([P, IC, 4, C], FP16, tag="xb_all")
            for g in range(IC):
                nc.sync.dma_start(out=x_all[:, g], in_=x_g[g])
                nc.vector.tensor_copy(out=xb_all[:, g], in_=x_all[:, g])
                for tt in range(4):
                    gt = 4 * g + tt
                    tq = nc.scalar if g < 2 else nc.sync
                    tq.dma_start(
                        out=xT[:, :, gt * P:(gt + 1) * P],
                        in_=xb_all[:, g, tt, :], transpose=True)

        # --- phase 2: packed QK projections + V^T (contract over c) ---
        def proj_chunk(pool, bufs, w_s, dst, ic, tag, part=P, eng=None):
            pp = pool.tile([part, NB], FP32, tag=tag, bufs=bufs)
            for ch in range(CCH):
                nc.tensor.matmul(
                    pp, lhsT=w_s[:, ch, 0:part],
                    rhs=xT[:, ch, ic * NB:(ic + 1) * NB],
                    start=(ch == 0), stop=(ch == CCH - 1))
            if eng is None:
                nc.vector.tensor_copy(out=dst[:, ic * NB:(ic + 1) * NB], in_=pp)
            else:
                eng.copy(out=dst[:, ic * NB:(ic + 1) * NB], in_=pp)

        with tc.tile_pool(name="p2psum", bufs=1, space="PSUM") as p2psum:
            for ic in range(IC):
                proj_chunk(p2psum, 2, wqk_b, qk_b, ic, "ppb")
            for ic in range(IC):
                proj_chunk(p2psum, 2, wqk_a, qk_a, ic, "ppa")
            for ic in range(IC):
                proj_chunk(p2psum, 2, wv_s, vT, ic, "ppv", part=DK, eng=nc.scalar)
            # V^T -> V tiles [128, 64] via PE transpose (col 64 stays ones)
            for tt in range(TT):
                pv = p2psum.tile([P, DK], FP32, tag="pv", bufs=2)
                nc.tensor.matmul(
                    pv, lhsT=vT[:, tt * P:(tt + 1) * P], rhs=idb[0:DK, 0:DK],
                    start=True, stop=True)
                nc.scalar.copy(out=v_s[:, tt, 0:DK], in_=pv)

        # --- main loop: row-packed S^T pairs -> split exp -> AV ---
        with (
            tc.tile_pool(name="spsum", bufs=1, space="PSUM") as spsum,
            tc.tile_pool(name="opsum", bufs=1, space="PSUM") as opsum,
            tc.tile_pool(name="ppool", bufs=3) as ppool,
            tc.tile_pool(name="outp", bufs=2) as outp,
        ):
            o_ps = []
            for ic in range(IC):
                o_tile = opsum.tile([DK + 1, NB], FP32, tag=f"ops{ic}")
                o_ps.append(o_tile)
            def emit_epilogue(qc):
                # PE transpose-back of out^T, normalize, batched store;
                # emitted right after the last AV of this q-chunk so it
                # hides under the remaining pairs' compute
                nc.vector.tensor_copy(out=oT80[0:DK + 1, qc, :], in_=o_ps[qc])
                o_big = outp.tile([P, 4, DK], FP32, tag="obig", bufs=2)
                for t4 in range(4):
                    otp = spsum.tile([P, 2, NB], FP32, tag="spair", bufs=2)
                    nc.tensor.matmul(
                        otp[:, 0, 0:DK + 1],
                        lhsT=oT80[0:DK + 1, qc, t4 * P:(t4 + 1) * P],
                        rhs=idb[0:DK + 1, 0:DK + 1],
                        start=True, stop=True, skip_group_check=True)
                    recip = outp.tile([P, 1], FP32, tag="recip", bufs=4)
                    nc.vector.reciprocal(recip, otp[:, 0, DK:DK + 1])
                    nc.vector.tensor_scalar_mul(
                        o_big[:, t4, :], otp[:, 0, 0:DK], recip)
                nc.sync.dma_start(
                    out=out_q[qc].rearrange("(t4 p) d -> p t4 d", p=P),
                    in_=o_big)

            for p_i in range(NP):
                j0, j1 = 2 * p_i, 2 * p_i + 1
                for qc in range(IC):
                    # two PSUM banks, one tile: [:,0,:] = j0, [:,1,:] = j1
                    s_pair = spsum.tile([P, 2, NB], FP32, tag="spair", bufs=2)
                    nc.tensor.matmul(
                        s_pair[:, 0, :],
                        lhsT=qk_b[0:DK, j0 * P:(j0 + 1) * P],
                        rhs=qk_a[0:DK, qc * NB:(qc + 1) * NB],
                        start=True, stop=True, skip_group_check=True)
                    nc.tensor.matmul(
                        s_pair[:, 1, :],
                        lhsT=qk_a[DK:P, j1 * P:(j1 + 1) * P],
                        rhs=qk_b[DK:P, qc * NB:(qc + 1) * NB],
                        start=True, stop=True, skip_group_check=True)
                    pp_t = ppool.tile([P, 2, NB], FP16, tag="ppt")
                    nc.scalar.activation(
                        out=pp_t[:, :, 0:SPL], in_=s_pair[:, :, 0:SPL],
                        func=mybir.ActivationFunctionType.Exp, scale=float(SCALE))
                    # fp16 Schraudolph straight into the fp16 tile (no copy)
                    nc.vector.tensor_scalar(
                        out=pp_t[:, :, SPL:NB].bitcast(I16),
                        in0=s_pair[:, :, SPL:NB],
                        scalar1=A_SCH, scalar2=B_SCH,
                        op0=mybir.AluOpType.mult, op1=mybir.AluOpType.add)
                    for jj, j in ((0, j0), (1, j1)):
                        nc.tensor.matmul(
                            o_ps[qc], lhsT=v_s[:, j, :], rhs=pp_t[:, jj, :],
                            start=(p_i == 0 and jj == 0),
                            stop=(p_i == NP - 1 and jj == 1),
                            skip_group_check=True)

            for qc in range(IC):
                emit_epilogue(qc)

    nc.compile()
    return nc


def _get_nc():
    if "nc" not in _cached:
        _cached["nc"] = _build_nc()
    return _cached["nc"]


def kernel(x, Wq, Wk, Wv, **run_kwargs):
    x = np.asarray(x, dtype=np.float32)
    Wq = np.asarray(Wq, dtype=np.float32)
    Wk = np.asarray(Wk, dtype=np.float32)
    Wv = np.asarray(Wv, dtype=np.float32)
    nc = _get_nc()
    in_maps = [
        {"x": np.ascontiguousarray(x[b]), "Wq": Wq, "Wk": Wk, "Wv": Wv}
        for b in range(B)
    ]
    res = run_bass_kernel_spmd(nc, in_maps, list(range(N_CORES)), **run_kwargs)
    out = np.stack([res.results[b]["out"] for b in range(B)], axis=0)
    if run_kwargs:
        _cached["last_result"] = res
    return out


# revision 42
# speedup vs baseline: 1.0415x; 1.0415x over previous
"""Single-head attention (B=8, T=2048, C=512, d_k=64) on 8 Trainium2 cores.

Strategy: data-parallel over batch B — one batch element per NeuronCore,
no collectives. All matmuls fp16 (1 PE cycle/row) with fp32 PSUM
accumulation; transposes are regular PE matmuls against an fp16
identity.

S^T matmuls contract over d_k=64, which would leave half the 128x128 PE
array idle, so key-tiles are processed in PAIRS packed into disjoint
row-groups (tile_position (0,0) / (64,0)) and run concurrently — this
also keeps the HAM activity monitor happy so the PE holds 2.4 GHz in
the steady state. The packing needs Q^T/K^T replicated on both
partition halves, so the projections run twice with swapped stationary
weights [Wq|Wk] / [Wk|Wq], producing QK_A = [Q^T; K^T] and
QK_B = [K^T; Q^T] at no extra matmul cost.

Softmax is split: ACT does exact exp on 56% of each score tile; DVE
writes the rest directly as fp16 bits via the Schraudolph trick
(int16(A*x+B) ~= fp16 exp; ~2% weight error on 44% of keys -> ~0.6%
output error vs the 2e-2 budget). The pair's two PSUM banks are one
contiguous tile, so each engine covers both keys of the pair in a
single strided-AP instruction.

A ones column in V makes the softmax denominator fall out of the AV
matmul. The epilogue transposes out^T back via the DMA XBAR (padded to
80 rows), normalizes with DVE reciprocal/multiply, and stores each
512-token chunk with one contiguous 128KB DMA.
"""

import numpy as np
from contextlib import ExitStack

import concourse.bass as bass
import concourse.tile as tile
from concourse import bacc
from concourse import mybir
from concourse.bass_utils import run_bass_kernel_spmd
from concourse.masks import make_identity

B, T, C, DK = 8, 2048, 512, 64
N_CORES = 8
FP32 = mybir.dt.float32
FP16 = mybir.dt.float16
I16 = mybir.dt.int16
P = 128
TT = T // P      # 16 token tiles
NP = TT // 2     # 8 key-tile pairs
CCH = C // P     # 4 contraction chunks
NB = 512         # matmul moving-operand max (PSUM bank = 512 fp32)
IC = T // NB     # 4 i-chunks
SPL = 256        # per-key ACT exp columns (of NB); rest on DVE
PAD = 80         # out^T rows padded to /16 for the DMA XBAR transpose
SCALE = 1.0 / np.sqrt(np.float32(DK))
# fp16 Schraudolph: int16(A*x + B) bitcast to fp16 approximates exp(x)
A_SCH = float((1 << 10) / np.log(2.0) * SCALE)
B_SCH = float(15 * (1 << 10) - 366393.0 / (1 << 13))

_cached = {}


def _build_nc():
    nc = bacc.Bacc("TRN2", target_bir_lowering=False, debug=False)
    x_d = nc.declare_dram_parameter("x", [T, C], FP32, isOutput=False)
    wq_d = nc.declare_dram_parameter("Wq", [C, DK], FP32, isOutput=False)
    wk_d = nc.declare_dram_parameter("Wk", [C, DK], FP32, isOutput=False)
    wv_d = nc.declare_dram_parameter("Wv", [C, DK], FP32, isOutput=False)
    out_d = nc.declare_dram_parameter("out", [T, DK], FP32, isOutput=True)

    x_g = x_d.rearrange("(g tt p) c -> g p tt c", p=P, tt=4)   # [4,128,4,512]
    out_q = out_d.rearrange("(ic p4) d -> ic p4 d", p4=4 * P)  # [4,512,64]

    with ExitStack() as ctx:
        tc = ctx.enter_context(tile.TileContext(nc))
        const = ctx.enter_context(tc.tile_pool(name="const", bufs=1))

        idb = const.tile([P, P], FP16)
        make_identity(nc, idb)

        # --- weights: fp32 staging -> fp16 packed stationaries ---
        wqk_a = const.tile([P, CCH, P], FP16)   # [Wq | Wk] per c-chunk
        wqk_b = const.tile([P, CCH, P], FP16)   # [Wk | Wq]
        wv_s = const.tile([P, CCH, DK], FP16)
        with tc.tile_pool(name="wstage", bufs=1) as wstage:
            stg = {}
            for w_d in (wq_d, wk_d, wv_d):
                w_stg = wstage.tile([P, CCH, DK], FP32, tag=f"stg{w_d.name}")
                nc.sync.dma_start(out=w_stg, in_=w_d.rearrange("(ch p) d -> p ch d", p=P))
                stg[w_d.name] = w_stg
            nc.vector.tensor_copy(out=wqk_a[:, :, 0:DK], in_=stg["Wq"])
            nc.vector.tensor_copy(out=wqk_a[:, :, DK:P], in_=stg["Wk"])
            nc.vector.tensor_copy(out=wqk_b[:, :, 0:DK], in_=stg["Wk"])
            nc.vector.tensor_copy(out=wqk_b[:, :, DK:P], in_=stg["Wq"])
            nc.vector.tensor_copy(out=wv_s, in_=stg["Wv"])

        xT = const.tile([P, CCH, T], FP16)          # x^T, 16KB/part
        v_s = const.tile([P, TT, DK + 1], FP16)     # V with ones col
        nc.vector.memset(v_s, 1.0)
        qk_a = const.tile([P, T], FP16)             # [Q^T; K^T]
        qk_b = const.tile([P, T], FP16)             # [K^T; Q^T]
        vT = const.tile([DK, T], FP16)
        # out^T staging, padded to 80 rows for the XBAR; rows 65-79 = 0
        oT80 = const.tile([PAD, IC, NB], FP16)
        nc.vector.memset(oT80, 0.0)

        # --- phases interleaved per 512-token group: load/cast/PE-transpose
        #     4 tiles, then immediately project that chunk (keeps the PE
        #     stream dense so the HAM clock gate warms early) ---
        xbpool_cm = tc.tile_pool(name="xbpool", bufs=1)
        xbpool = xbpool_cm.__enter__()
        tpsum_cm = tc.tile_pool(name="tpsum", bufs=2, space="PSUM")
        tpsum = tpsum_cm.__enter__()
        x_all = xbpool.tile([P, IC, 4, C], FP32, tag="x_all")
        xb_all = xbpool.tile([P, IC, 4, C], FP16, tag="xb_all")
        for g in range(IC):
            nc.sync.dma_start(out=x_all[:, g], in_=x_g[g])
            nc.vector.tensor_copy(out=xb_all[:, g], in_=x_all[:, g])
            for tt in range(4):
                gt = 4 * g + tt
                tps = tpsum.tile([P, CCH, P], FP32, tag="tps", bufs=2)
                for ch in range(CCH):
                    nc.tensor.matmul(
                        tps[:, ch, :], lhsT=xb_all[:, g, tt, ch * P:(ch + 1) * P],
                        rhs=idb, start=True, stop=True)
                nc.scalar.copy(out=xT[:, :, gt * P:(gt + 1) * P], in_=tps)

            # project this 512-token chunk (both replicas + V^T)
            for (w_s, dst, part, eng) in (
                (wqk_b, qk_b, P, None), (wqk_a, qk_a, P, None),
                (wv_s, vT, DK, nc.scalar),
            ):
                pp = tpsum.tile([P, CCH, P], FP32, tag="pp", bufs=2)
                ppv = pp[0:part, 0, 0:NB] if False else pp.rearrange(
                    "p a b -> p (a b)")[0:part, 0:NB]
                for ch in range(CCH):
                    nc.tensor.matmul(
                        ppv, lhsT=w_s[:, ch, 0:part],
                        rhs=xT[:, ch, g * NB:(g + 1) * NB],
                        start=(ch == 0), stop=(ch == CCH - 1))
                if eng is None:
                    nc.vector.tensor_copy(out=dst[:, g * NB:(g + 1) * NB], in_=ppv)
                else:
                    eng.copy(out=dst[:, g * NB:(g + 1) * NB], in_=ppv)
            # V^T -> V tiles [128, 64] via PE transpose (col 64 stays ones)
            for tt in range(4):
                gt = 4 * g + tt
                pv = tpsum.tile([P, CCH, P], FP32, tag="tps", bufs=2)
                nc.tensor.matmul(
                    pv[:, 0, 0:DK], lhsT=vT[:, gt * P:(gt + 1) * P],
                    rhs=idb[0:DK, 0:DK], start=True, stop=True)
                nc.scalar.copy(out=v_s[:, gt, 0:DK], in_=pv[:, 0, 0:DK])
        tpsum_cm.__exit__(None, None, None)
        xbpool_cm.__exit__(None, None, None)

        # --- main loop: row-packed S^T pairs -> split exp -> AV ---
        with (
            tc.tile_pool(name="spsum", bufs=1, space="PSUM") as spsum,
            tc.tile_pool(name="opsum", bufs=1, space="PSUM") as opsum,
            tc.tile_pool(name="ppool", bufs=3) as ppool,
            tc.tile_pool(name="outp", bufs=2) as outp,
        ):
            o_ps = []
            for ic in range(IC):
                o_tile = opsum.tile([DK + 1, NB], FP32, tag=f"ops{ic}")
                o_ps.append(o_tile)
            def emit_epilogue(qc):
                # PE transpose-back of out^T, normalize, batched store;
                # emitted right after the last AV of this q-chunk so it
                # hides under the remaining pairs' compute
                nc.vector.tensor_copy(out=oT80[0:DK + 1, qc, :], in_=o_ps[qc])
                o_big = outp.tile([P, 4, DK], FP32, tag="obig", bufs=2)
                for t4 in range(4):
                    otp = spsum.tile([P, 2, NB], FP32, tag="spair", bufs=2)
                    nc.tensor.matmul(
                        otp[:, 0, 0:DK + 1],
                        lhsT=oT80[0:DK + 1, qc, t4 * P:(t4 + 1) * P],
                        rhs=idb[0:DK + 1, 0:DK + 1],
                        start=True, stop=True, skip_group_check=True)
                    recip = outp.tile([P, 1], FP32, tag="recip", bufs=4)
                    nc.vector.reciprocal(recip, otp[:, 0, DK:DK + 1])
                    nc.vector.tensor_scalar_mul(
                        o_big[:, t4, :], otp[:, 0, 0:DK], recip)
                nc.sync.dma_start(
                    out=out_q[qc].rearrange("(t4 p) d -> p t4 d", p=P),
                    in_=o_big)

            for p_i in range(NP):
                j0, j1 = 2 * p_i, 2 * p_i + 1
                for qc in range(IC):
                    # two PSUM banks, one tile: [:,0,:] = j0, [:,1,:] = j1
                    s_pair = spsum.tile([P, 2, NB], FP32, tag="spair", bufs=2)
                    nc.tensor.matmul(
                        s_pair[:, 0, :],
                        lhsT=qk_b[0:DK, j0 * P:(j0 + 1) * P],
                        rhs=qk_a[0:DK, qc * NB:(qc + 1) * NB],
                        start=True, stop=True, skip_group_check=True)
                    nc.tensor.matmul(
                        s_pair[:, 1, :],
                        lhsT=qk_a[DK:P, j1 * P:(j1 + 1) * P],
                        rhs=qk_b[DK:P, qc * NB:(qc + 1) * NB],
                        start=True, stop=True, skip_group_check=True)
                    pp_t = ppool.tile([P, 2, NB], FP16, tag="ppt")
                    nc.scalar.activation(
                        out=pp_t[:, :, 0:SPL], in_=s_pair[:, :, 0:SPL],
                        func=mybir.ActivationFunctionType.Exp, scale=float(SCALE))
                    # fp16 Schraudolph straight into the fp16 tile (no copy)
                    nc.vector.tensor_scalar(
                        out=pp_t[:, :, SPL:NB].bitcast(I16),
                        in0=s_pair[:, :, SPL:NB],
                        scalar1=A_SCH, scalar2=B_SCH,
                        op0=mybir.AluOpType.mult, op1=mybir.AluOpType.add)
                    for jj, j in ((0, j0), (1, j1)):
                        nc.tensor.matmul(
                            o_ps[qc], lhsT=v_s[:, j, :], rhs=pp_t[:, jj, :],
                            start=(p_i == 0 and jj == 0),
                            stop=(p_i == NP - 1 and jj == 1),
                            skip_group_check=True)

            for qc in range(IC):
                emit_epilogue(qc)

    nc.compile()
    return nc


def _get_nc():
    if "nc" not in _cached:
        _cached["nc"] = _build_nc()
    return _cached["nc"]


def kernel(x, Wq, Wk, Wv, **run_kwargs):
    x = np.asarray(x, dtype=np.float32)
    Wq = np.asarray(Wq, dtype=np.float32)
    Wk = np.asarray(Wk, dtype=np.float32)
    Wv = np.asarray(Wv, dtype=np.float32)
    nc = _get_nc()
    in_maps = [
        {"x": np.ascontiguousarray(x[b]), "Wq": Wq, "Wk": Wk, "Wv": Wv}
        for b in range(B)
    ]
    res = run_bass_kernel_spmd(nc, in_maps, list(range(N_CORES)), **run_kwargs)
    out = np.stack([res.results[b]["out"] for b in range(B)], axis=0)
    if run_kwargs:
        _cached["last_result"] = res
    return out
